# revision 7
# baseline (speedup 1.0000x reference)
"""MinLSTM cell (Heinsen-scan reference) as a Bass/Tile kernel for 8 trn2 NeuronCores.

Linear-space rewrite of the reference's log-space scan:
    h_t = f'_t h_{t-1} + (1 - f'_t) g(pre_h_t),   h_0 = 1e-6
with f' = sigmoid(pre_f+b_f) / (sigmoid(pre_f+b_f) + sigmoid(pre_i+b_i)) and
g(x) = max(x+0.5, sigmoid(x)).

Distribution: data-parallel over batch N=8, one batch element per core, W/b
replicated. Device layout: channels on SBUF partitions (4 c-tiles of 128),
sequence along the free dim. Chunks are emitted round-robin across c-tiles
so each c-tile's sequential scan chain has three other chunks of work
between dependent scans.

Per-core engine assignment:
  PE : F/I gate matmuls in fp8 E4M3 (x*16, W*64) with DoubleRow perf mode,
       H gate matmul in bf16 (fp16 measures 427ns/512col on HW vs bf16's
       ~216 - fp16 runs at half rate).
  ACT: sf = sigmoid(psF/1024 + b_f); si = sigmoid(psI/1024 + b_i);
       sh = sigmoid(psH + b_h); rl = relu(psH + b_h + 0.5). PSUM is drained
       by ACT alone so the psum rotation is a pure PE<->ACT loop.
  DVE: fp = FRACT_FAST_ANT(sf, si) = sf/(sf+si) fused custom op;
       htl = max(rl, sh) (2x-mode tensor_tensor; exact g identity
       g(x) = max(relu(x+0.5), sigmoid(x)));
       scans h = tensor_tensor_scan(fp, wv, mult, subtract) as 2 sub-scans
       per chunk, deferred 1-2 slots for pipeline depth.
  DVE also: fpm1 = fp - 1 (tensor_scalar, 4x mode); then wv is formed by
       a SWDGE accumulate-mult DMA (fpm1 *= htl) so the multiply runs on
       the DMA engines, off every compute engine. GPSIMD tensor_tensor is
       deliberately unused: concurrent Pool-engine SBUF traffic inflates
       the DVE scan's loop-carried latency ~2.6x (measured).
  GPS: SWDGE descriptor generation only.
  SP : all HBM loads/stores.
"""

import os
import sys

import numpy as np

sys.path.insert(0, "/opt/trn_rl_repo")

import ml_dtypes  # noqa: E402

import concourse.bass as bass  # noqa: E402
import concourse.tile as tile  # noqa: E402
from concourse import bacc, mybir  # noqa: E402
from concourse import dve_ops  # noqa: E402
from concourse.dve_spec import (  # noqa: E402
    AluOp,
    Bin,
    C0,
    C1,
    Spec,
    lower,
)
from concourse.dve_uop import DveOpSpec  # noqa: E402

N, L, H_IN, H = 8, 4096, 512, 512
P = 128
NK = H_IN // P  # 4 k-blocks of the contraction dim
NCT = H // P  # 4 channel tiles
LT = 512  # matmul moving tile (one PSUM bank of fp32)
F32 = mybir.dt.float32
F16 = mybir.dt.float16
BF16 = mybir.dt.bfloat16
F8 = mybir.dt.float8e4
Alu = mybir.AluOpType
Act = mybir.ActivationFunctionType
DR = mybir.MatmulPerfMode.DoubleRow

HX_INIT = 1e-6
XS, WS = 16.0, 64.0  # fp8 scale for x and W (TRN E4M3 max is +-240)
FR_C0, FR_C1 = -0.23549792, 2.0017324  # recip bit-seed Chebyshev consts

# chunk column-lists per c-tile
CLIST = [2048, 2048]
# sub-scan split within a chunk (pipeline depth without extra ACT instrs)
SSPLIT = 1024

_cached_nc = {}
_fract_op = None


def _register_dve_ops():
    """Author + register the fused f' = sf/(sf+si) DVE op (bit-trick recip
    seed + one Newton step + multiply, ~0.17% max rel err)."""
    global _fract_op
    if _fract_op is not None:
        return _fract_op

    def _np_recip_seed_nr1(s, c0, c1):
        ns = (~s.view(np.int32)).view(np.float32)
        y0 = ns * c0
        return y0 * (c1 - s * y0)

    def _ref_fract(in0, in1, c0, c1, c2):
        sf = in0.astype(np.float32)
        s = sf + in1.astype(np.float32)
        return sf * _np_recip_seed_nr1(s, c0, c1)

    from concourse.dve_spec import Src0, Src1  # noqa: E402

    s_expr = Src0 + Src1
    not_s = Bin(AluOp.BITWISE_NOT, s_expr, s_expr)
    y0 = not_s * C0
    y1 = y0 * (C1 - s_expr * y0)
    fract_spec = Spec(body=Src0 * y1, reference=_ref_fract)

    name = "FRACT_FAST_ANT"
    existing = next((o for o in dve_ops.OPS if o.name == name), None)
    if existing is not None:
        _fract_op = existing
        return _fract_op
    row = dve_ops._CUSTOM_DVE_ROW_BASE + len(dve_ops.OPS)
    shas = {}
    for ver in ("v3",):
        tmp = DveOpSpec(
            name=name,
            opcode=row,
            uops=lower(fract_spec, ver=ver),
            rd1_en=True,
        )
        shas[ver] = tmp.sha(ver)
    op = dve_ops.DveOp(name=name, spec=fract_spec, subdim=False, uops_sha=shas)
    dve_ops.OPS.append(op)
    dve_ops._SUB_OPCODE_FOR_NAME[name] = row
    dve_ops.CUSTOM_DVE_SPECS[name] = fract_spec
    _fract_op = op
    return _fract_op


def build_program():
    key = 0
    if key in _cached_nc:
        return _cached_nc[key]
    fract_op = _register_dve_ops()

    nc = bacc.Bacc()
    xTbf_d = nc.dram_tensor("xTbf", [H_IN, L], BF16, kind="ExternalInput")
    xT8_d = nc.dram_tensor("xT8", [H_IN, L], F8, kind="ExternalInput")
    wbf_d = nc.dram_tensor("wbf", [H_IN, NCT * P], BF16, kind="ExternalInput")
    w8_d = nc.dram_tensor("w8", [H_IN, NCT * 2 * P], F8, kind="ExternalInput")
    bias_d = nc.dram_tensor("bias", [P, 16], F32, kind="ExternalInput")
    out_d = nc.dram_tensor("out", [H, L], F16, kind="ExternalOutput")

    CW8 = 2 * P  # fp8 weight cols per c-tile: [F_c | I_c]
    LMAX = max(CLIST)

    with tile.TileContext(nc) as tc:
        with (
            tc.tile_pool(name="const", bufs=1) as const_pool,
            tc.tile_pool(name="gates", bufs=3) as gates_pool,
            tc.tile_pool(name="scanbuf", bufs=1) as scan_pool,
            tc.tile_pool(name="psum", bufs=2, space="PSUM") as psum_pool,
        ):
            # Warmup activation: absorbs the one-time sigmoid act-table load.
            warm = const_pool.tile([P, 8], F32)
            nc.vector.memset(warm[:], 0.0)
            nc.scalar.activation(warm[:], warm[:], Act.Sigmoid)
            # PE warmup: garbage matmuls with no deps so the HAM clock gate
            # reaches 2.4GHz while the first DMAs are in flight.
            wup = const_pool.tile([P, P], BF16)
            nc.vector.memset(wup[:], 0.0)
            wup_ps = psum_pool.tile([P, P], F32, tag="ps")
            for _ in range(16):
                nc.tensor.matmul(wup_ps[:], wup[:], wup[:], start=True, stop=True)

            xTbf_sb = const_pool.tile([P, NK, L], BF16)
            xT8_sb = const_pool.tile([P, NK, L], F8)
            wbf_sb = const_pool.tile([P, NK, NCT * P], BF16)
            w8_sb = const_pool.tile([P, NK, NCT * CW8], F8)
            bias_sb = const_pool.tile([P, 16], F32)

            w8_r = w8_d.rearrange("(ki p) o -> p ki o", p=P)
            wbf_r = wbf_d.rearrange("(ki p) o -> p ki o", p=P)
            xTbf_r = xTbf_d.rearrange("(ki p) l -> p ki l", p=P)
            xT8_r = xT8_d.rearrange("(ki p) l -> p ki l", p=P)

            # Load order prioritizes the first round-robin sweep (j=0, 1024
            # cols, all four c-tiles): w8[c0], x8 head, wbf[c0], bias, xbf
            # head, the other c-tiles' weights, then the remaining x chunks.
            xsplits = []
            off = 0
            for sz in CLIST:
                xsplits.append((off, off + sz))
                off += sz

            nc.sync.dma_start(w8_sb[:, :, 0:CW8], w8_r[:, :, 0:CW8])
            s0, e0 = xsplits[0]
            # First-chunk x loads in 512-col pieces: the first 512-col matmul
            # group only needs the first piece, so PE starts ~6us in instead
            # of waiting for the full first chunk.
            nc.sync.dma_start(xT8_sb[:, :, s0 : s0 + 512], xT8_r[:, :, s0 : s0 + 512])
            nc.sync.dma_start(wbf_sb[:, :, 0:P], wbf_r[:, :, 0:P])
            nc.sync.dma_start(bias_sb[:], bias_d[:])
            nc.sync.dma_start(xTbf_sb[:, :, s0 : s0 + 512], xTbf_r[:, :, s0 : s0 + 512])
            for p in range(s0 + 512, e0, 512):
                nc.sync.dma_start(xT8_sb[:, :, p : p + 512], xT8_r[:, :, p : p + 512])
                nc.sync.dma_start(xTbf_sb[:, :, p : p + 512], xTbf_r[:, :, p : p + 512])
            for cg in range(1, NCT):
                nc.sync.dma_start(
                    w8_sb[:, :, cg * CW8 : (cg + 1) * CW8],
                    w8_r[:, :, cg * CW8 : (cg + 1) * CW8],
                )
                nc.sync.dma_start(
                    wbf_sb[:, :, cg * P : (cg + 1) * P],
                    wbf_r[:, :, cg * P : (cg + 1) * P],
                )
            for s, e in xsplits[1:]:
                nc.sync.dma_start(xT8_sb[:, :, s:e], xT8_r[:, :, s:e])
                nc.sync.dma_start(xTbf_sb[:, :, s:e], xTbf_r[:, :, s:e])

            hvs = {
                c: scan_pool.tile([P, L], F16, tag=f"hv{c}", name=f"hv{c}")
                for c in range(NCT)
            }

            order = [(c, j) for j in range(len(CLIST)) for c in range(NCT)]

            # Sub-scan deferral: chunk k's first sub-scan is emitted with
            # chunk k+1's head, its second with chunk k+2's head. The GPS
            # fpm1/wv pair is emitted inline (lag-0) - the GPS queue itself
            # provides buffering since nothing else runs there.
            pend_scans = []  # flat list of (c, ls, Lsub, fp, fp_off, wv)

            def emit_subscan(item):
                c, ls, Lsub, fp, off, wv = item
                hv = hvs[c]
                init = HX_INIT if ls == 0 else hv[:, ls - 1 : ls]
                nc.vector.tensor_tensor_scan(
                    hv[:, ls : ls + Lsub], fp[:, off : off + Lsub],
                    wv[:, off : off + Lsub], init,
                    Alu.mult, Alu.subtract,
                )
                nc.sync.dma_start(
                    out_d[c * P : (c + 1) * P, ls : ls + Lsub],
                    hv[:, ls : ls + Lsub],
                )

            lsoff = {c: 0 for c in range(NCT)}
            for c, j in order:
                LHC = CLIST[j]
                ls = lsoff[c]

                sf = gates_pool.tile([P, LHC], F16, tag="sf")
                si = gates_pool.tile([P, LHC], F16, tag="si")
                sh = gates_pool.tile([P, LHC], F16, tag="sh")
                fp = gates_pool.tile([P, LHC], F16, tag="fp")

                def mms8(ps, ocol):
                    for j2 in range(LHC // LT):
                        xk = slice(ls + j2 * LT, ls + (j2 + 1) * LT)
                        jl = slice(j2 * LT, (j2 + 1) * LT)
                        for kp in range(NK // 2):
                            nc.tensor.matmul(
                                ps[:, jl],
                                w8_sb[:, 2 * kp : 2 * kp + 2, ocol : ocol + P],
                                xT8_sb[:, 2 * kp : 2 * kp + 2, xk],
                                start=kp == 0,
                                stop=kp == NK // 2 - 1,
                                perf_mode=DR,
                            )

                def mmsbf(ps, ocol):
                    for j2 in range(LHC // LT):
                        xk = slice(ls + j2 * LT, ls + (j2 + 1) * LT)
                        jl = slice(j2 * LT, (j2 + 1) * LT)
                        for ki in range(NK):
                            nc.tensor.matmul(
                                ps[:, jl],
                                wbf_sb[:, ki, ocol : ocol + P],
                                xTbf_sb[:, ki, xk],
                                start=ki == 0,
                                stop=ki == NK - 1,
                            )

                # F gate
                psF = psum_pool.tile([P, LHC], F32, tag="ps")
                mms8(psF, c * CW8)
                nc.scalar.activation(
                    sf[:], psF[:], Act.Sigmoid,
                    bias=bias_sb[:, 0 * NCT + c : 0 * NCT + c + 1],
                    scale=1.0 / (XS * WS),
                )
                # I gate
                psI = psum_pool.tile([P, LHC], F32, tag="ps")
                mms8(psI, c * CW8 + P)
                nc.scalar.activation(
                    si[:], psI[:], Act.Sigmoid,
                    bias=bias_sb[:, 1 * NCT + c : 1 * NCT + c + 1],
                    scale=1.0 / (XS * WS),
                )
                # H gate
                psH = psum_pool.tile([P, LHC], F32, tag="ps")
                mmsbf(psH, c * P)
                nc.scalar.activation(
                    sh[:], psH[:], Act.Sigmoid,
                    bias=bias_sb[:, 2 * NCT + c : 2 * NCT + c + 1],
                )
                rl = gates_pool.tile([P, LHC], F16, tag="rl")
                nc.scalar.activation(
                    rl[:], psH[:], Act.Relu,
                    bias=bias_sb[:, 3 * NCT + c : 3 * NCT + c + 1],
                )
                # f' = sf/(sf+si), one fused DVE op over the whole chunk
                nc.vector._custom_dve(
                    fract_op, out=fp[:], in0=sf[:], in1=si[:],
                    s0=FR_C0, s1=FR_C1,
                )
                # htil = max(relu(psH+b_h+0.5), sigmoid(psH+b_h)): exact g,
                # 2x-mode tensor_tensor max in-place into rl.
                nc.vector.tensor_tensor(rl[:], rl[:], sh[:], Alu.max)
                fpm1 = gates_pool.tile([P, LHC], F16, tag="fpm1")
                nc.vector.tensor_scalar_add(fpm1[:], fp[:], -1.0)
                wv = gates_pool.tile([P, LHC], F16, tag="wv")
                nc.vector.tensor_tensor(wv[:], fpm1[:], rl[:], Alu.mult)

                for off in range(0, LHC, SSPLIT):
                    Lsub = min(SSPLIT, LHC - off)
                    pend_scans.append((c, ls + off, Lsub, fp, off, wv))
                # Drain sub-scans at one-per-slot-ish pace: keep at most
                # 2 chunks' worth (4 sub-scans) pending.
                while len(pend_scans) > 4:
                    emit_subscan(pend_scans.pop(0))
                lsoff[c] += LHC

            while pend_scans:
                emit_subscan(pend_scans.pop(0))

    nc.compile()
    _cached_nc[key] = nc
    return nc


def reorder_w8(W: np.ndarray) -> np.ndarray:
    """fp8 weights, [H_IN, NCT*2P] with per-c-tile [F_c | I_c] col groups."""
    Wf, Wi = W[:H], W[H : 2 * H]
    cols = []
    for c in range(NCT):
        cols.append(Wf[c * P : (c + 1) * P])
        cols.append(Wi[c * P : (c + 1) * P])
    w = np.concatenate(cols, axis=0)  # [NCT*2P, H_IN]
    w8 = np.clip(w.T * WS, -240.0, 240.0).astype(ml_dtypes.float8_e4m3fn)
    return np.ascontiguousarray(w8)


def reorder_w16(W: np.ndarray) -> np.ndarray:
    Wh = W[2 * H :]
    return np.ascontiguousarray(Wh.T.astype(ml_dtypes.bfloat16))


def make_bias(b: np.ndarray) -> np.ndarray:
    b32 = np.asarray(b, dtype=np.float32)
    bias = np.empty((P, 16), dtype=np.float32)
    for c in range(NCT):
        bias[:, 0 * NCT + c] = b32[0 * H + c * P : 0 * H + (c + 1) * P]
        bias[:, 1 * NCT + c] = b32[1 * H + c * P : 1 * H + (c + 1) * P]
        bias[:, 2 * NCT + c] = b32[2 * H + c * P : 2 * H + (c + 1) * P]
        bias[:, 3 * NCT + c] = b32[2 * H + c * P : 2 * H + (c + 1) * P] + 0.5
    return bias


def _prep_core_inputs(x_n, wbf, w8, bias):
    xT = np.ascontiguousarray(x_n.T)
    return {
        "xTbf": xT.astype(ml_dtypes.bfloat16),
        "xT8": np.clip(xT * XS, -240.0, 240.0).astype(ml_dtypes.float8_e4m3fn),
        "wbf": wbf,
        "w8": w8,
        "bias": bias,
    }


def kernel(x: np.ndarray, W: np.ndarray, b: np.ndarray) -> np.ndarray:
    from concourse.bass_utils import run_bass_kernel_spmd

    nc = build_program()

    W = np.asarray(W)
    wbf = reorder_w16(W)
    w8 = reorder_w8(W)
    bias = make_bias(b)

    in_maps = [_prep_core_inputs(np.asarray(x[n]), wbf, w8, bias) for n in range(N)]
    res = run_bass_kernel_spmd(nc, in_maps, list(range(N)))

    out = np.empty((N, L, H), dtype=np.float32)
    for n in range(N):
        out[n] = res.results[n]["out"].T.astype(np.float32)
    return out


# revision 9
# speedup vs baseline: 1.0324x; 1.0324x over previous
"""MinLSTM cell (Heinsen-scan reference) as a Bass/Tile kernel for 8 trn2 NeuronCores.

Linear-space rewrite of the reference's log-space scan:
    h_t = f'_t h_{t-1} + (1 - f'_t) g(pre_h_t),   h_0 = 1e-6
with f' = sigmoid(pre_f+b_f) / (sigmoid(pre_f+b_f) + sigmoid(pre_i+b_i)) and
g(x) = max(x+0.5, sigmoid(x)).

Distribution: data-parallel over batch N=8, one batch element per core, W/b
replicated. Device layout: channels on SBUF partitions (4 c-tiles of 128),
sequence along the free dim. Chunks are emitted round-robin across c-tiles
so each c-tile's sequential scan chain has three other chunks of work
between dependent scans.

Per-core engine assignment:
  PE : F/I gate matmuls in fp8 E4M3 (x*16, W*64) with DoubleRow perf mode,
       H gate matmul in bf16 (fp16 measures 427ns/512col on HW vs bf16's
       ~216 - fp16 runs at half rate).
  ACT: sf = sigmoid(psF/1024 + b_f); si = sigmoid(psI/1024 + b_i);
       sh = sigmoid(psH + b_h); rl = relu(psH + b_h + 0.5). PSUM is drained
       by ACT alone so the psum rotation is a pure PE<->ACT loop.
  DVE: fp = FRACT_FAST_ANT(sf, si) = sf/(sf+si) fused custom op;
       htl = max(rl, sh) (2x-mode tensor_tensor; exact g identity
       g(x) = max(relu(x+0.5), sigmoid(x)));
       scans h = tensor_tensor_scan(fp, wv, mult, subtract) as 2 sub-scans
       per chunk, deferred 1-2 slots for pipeline depth.
  DVE also: fpm1 = fp - 1 (tensor_scalar, 4x mode); then wv is formed by
       a SWDGE accumulate-mult DMA (fpm1 *= htl) so the multiply runs on
       the DMA engines, off every compute engine. GPSIMD tensor_tensor is
       deliberately unused: concurrent Pool-engine SBUF traffic inflates
       the DVE scan's loop-carried latency ~2.6x (measured).
  GPS: SWDGE descriptor generation only.
  SP : all HBM loads/stores.
"""

import os
import sys

import numpy as np

sys.path.insert(0, "/opt/trn_rl_repo")

import ml_dtypes  # noqa: E402

import concourse.bass as bass  # noqa: E402
import concourse.tile as tile  # noqa: E402
from concourse import bacc, mybir  # noqa: E402
from concourse import dve_ops  # noqa: E402
from concourse.dve_spec import (  # noqa: E402
    AluOp,
    Bin,
    C0,
    C1,
    Spec,
    lower,
)
from concourse.dve_uop import DveOpSpec  # noqa: E402

N, L, H_IN, H = 8, 4096, 512, 512
P = 128
NK = H_IN // P  # 4 k-blocks of the contraction dim
NCT = H // P  # 4 channel tiles
LT = 512  # matmul moving tile (one PSUM bank of fp32)
F32 = mybir.dt.float32
F16 = mybir.dt.float16
BF16 = mybir.dt.bfloat16
F8 = mybir.dt.float8e4
Alu = mybir.AluOpType
Act = mybir.ActivationFunctionType
DR = mybir.MatmulPerfMode.DoubleRow

HX_INIT = 1e-6
XS, WS = 16.0, 64.0  # fp8 scale for x and W (TRN E4M3 max is +-240)
FR_C0, FR_C1 = -0.23549792, 2.0017324  # recip bit-seed Chebyshev consts

# chunk column-lists per c-tile: small first round so the pipeline
# starts as soon as the first 128KB of x8 lands
CLIST = [512, 2048, 1536]
# sub-scan split within a chunk (pipeline depth without extra ACT instrs)
SSPLIT = 1024

_cached_nc = {}
_fract_op = None


def _register_dve_ops():
    """Author + register the fused f' = sf/(sf+si) DVE op (bit-trick recip
    seed + one Newton step + multiply, ~0.17% max rel err)."""
    global _fract_op
    if _fract_op is not None:
        return _fract_op

    def _np_recip_seed_nr1(s, c0, c1):
        ns = (~s.view(np.int32)).view(np.float32)
        y0 = ns * c0
        return y0 * (c1 - s * y0)

    def _ref_fract(in0, in1, c0, c1, c2):
        sf = in0.astype(np.float32)
        s = sf + in1.astype(np.float32)
        return sf * _np_recip_seed_nr1(s, c0, c1)

    from concourse.dve_spec import Src0, Src1  # noqa: E402

    s_expr = Src0 + Src1
    not_s = Bin(AluOp.BITWISE_NOT, s_expr, s_expr)
    y0 = not_s * C0
    y1 = y0 * (C1 - s_expr * y0)
    fract_spec = Spec(body=Src0 * y1, reference=_ref_fract)

    name = "FRACT_FAST_ANT"
    existing = next((o for o in dve_ops.OPS if o.name == name), None)
    if existing is not None:
        _fract_op = existing
        return _fract_op
    row = dve_ops._CUSTOM_DVE_ROW_BASE + len(dve_ops.OPS)
    shas = {}
    for ver in ("v3",):
        tmp = DveOpSpec(
            name=name,
            opcode=row,
            uops=lower(fract_spec, ver=ver),
            rd1_en=True,
        )
        shas[ver] = tmp.sha(ver)
    op = dve_ops.DveOp(name=name, spec=fract_spec, subdim=False, uops_sha=shas)
    dve_ops.OPS.append(op)
    dve_ops._SUB_OPCODE_FOR_NAME[name] = row
    dve_ops.CUSTOM_DVE_SPECS[name] = fract_spec
    _fract_op = op
    return _fract_op


def build_program():
    key = 0
    if key in _cached_nc:
        return _cached_nc[key]
    fract_op = _register_dve_ops()

    nc = bacc.Bacc()
    xTbf_d = nc.dram_tensor("xTbf", [H_IN, L], BF16, kind="ExternalInput")
    xT8_d = nc.dram_tensor("xT8", [H_IN, L], F8, kind="ExternalInput")
    wbf_d = nc.dram_tensor("wbf", [H_IN, NCT * P], BF16, kind="ExternalInput")
    w8_d = nc.dram_tensor("w8", [H_IN, NCT * 2 * P], F8, kind="ExternalInput")
    bias_d = nc.dram_tensor("bias", [P, 16], F32, kind="ExternalInput")
    out_d = nc.dram_tensor("out", [H, L], F16, kind="ExternalOutput")

    CW8 = 2 * P  # fp8 weight cols per c-tile: [F_c | I_c]
    LMAX = max(CLIST)

    with tile.TileContext(nc) as tc:
        with (
            tc.tile_pool(name="const", bufs=1) as const_pool,
            tc.tile_pool(name="gates", bufs=3) as gates_pool,
            tc.tile_pool(name="scanbuf", bufs=1) as scan_pool,
            tc.tile_pool(name="psum", bufs=2, space="PSUM") as psum_pool,
        ):
            # Warmup activation: absorbs the one-time sigmoid act-table load.
            warm = const_pool.tile([P, 8], F32)
            nc.vector.memset(warm[:], 0.0)
            nc.scalar.activation(warm[:], warm[:], Act.Sigmoid)
            # PE warmup: garbage matmuls with no deps so the HAM clock gate
            # reaches 2.4GHz while the first DMAs are in flight.
            wup = const_pool.tile([P, P], BF16)
            nc.vector.memset(wup[:], 0.0)
            wup_ps = psum_pool.tile([P, 512], F32, tag="ps")
            wupx = const_pool.tile([P, 512], BF16)
            nc.vector.memset(wupx[:], 0.0)
            for _ in range(24):
                nc.tensor.matmul(wup_ps[:], wup[:], wupx[:], start=True, stop=True)

            xTbf_sb = const_pool.tile([P, NK, L], BF16)
            xT8_sb = const_pool.tile([P, NK, L], F8)
            wbf_sb = const_pool.tile([P, NK, NCT * P], BF16)
            w8_sb = const_pool.tile([P, NK, NCT * CW8], F8)
            bias_sb = const_pool.tile([P, 16], F32)

            w8_r = w8_d.rearrange("(ki p) o -> p ki o", p=P)
            wbf_r = wbf_d.rearrange("(ki p) o -> p ki o", p=P)
            xTbf_r = xTbf_d.rearrange("(ki p) l -> p ki l", p=P)
            xT8_r = xT8_d.rearrange("(ki p) l -> p ki l", p=P)

            # Load order prioritizes the first round-robin sweep (j=0, 1024
            # cols, all four c-tiles): w8[c0], x8 head, wbf[c0], bias, xbf
            # head, the other c-tiles' weights, then the remaining x chunks.
            xsplits = []
            off = 0
            for sz in CLIST:
                xsplits.append((off, off + sz))
                off += sz

            nc.sync.dma_start(w8_sb[:, :, 0:CW8], w8_r[:, :, 0:CW8])
            s0, e0 = xsplits[0]
            # First-chunk x loads in 512-col pieces: the first 512-col matmul
            # group only needs the first piece, so PE starts ~6us in instead
            # of waiting for the full first chunk.
            nc.sync.dma_start(xT8_sb[:, :, s0 : s0 + 512], xT8_r[:, :, s0 : s0 + 512])
            nc.sync.dma_start(wbf_sb[:, :, 0:P], wbf_r[:, :, 0:P])
            nc.sync.dma_start(bias_sb[:], bias_d[:])
            nc.sync.dma_start(xTbf_sb[:, :, s0 : s0 + 512], xTbf_r[:, :, s0 : s0 + 512])
            for p in range(s0 + 512, e0, 512):
                nc.sync.dma_start(xT8_sb[:, :, p : p + 512], xT8_r[:, :, p : p + 512])
                nc.sync.dma_start(xTbf_sb[:, :, p : p + 512], xTbf_r[:, :, p : p + 512])
            for cg in range(1, NCT):
                nc.sync.dma_start(
                    w8_sb[:, :, cg * CW8 : (cg + 1) * CW8],
                    w8_r[:, :, cg * CW8 : (cg + 1) * CW8],
                )
                nc.sync.dma_start(
                    wbf_sb[:, :, cg * P : (cg + 1) * P],
                    wbf_r[:, :, cg * P : (cg + 1) * P],
                )
            for s, e in xsplits[1:]:
                nc.sync.dma_start(xT8_sb[:, :, s:e], xT8_r[:, :, s:e])
                nc.sync.dma_start(xTbf_sb[:, :, s:e], xTbf_r[:, :, s:e])

            hvs = {
                c: scan_pool.tile([P, L], F16, tag=f"hv{c}", name=f"hv{c}")
                for c in range(NCT)
            }

            order = [(c, j) for j in range(len(CLIST)) for c in range(NCT)]

            # Sub-scan deferral: chunk k's first sub-scan is emitted with
            # chunk k+1's head, its second with chunk k+2's head. The GPS
            # fpm1/wv pair is emitted inline (lag-0) - the GPS queue itself
            # provides buffering since nothing else runs there.
            pend_scans = []  # flat list of (c, ls, Lsub, fp, fp_off, wv)
            pend_wv = []  # (c, ls, LHC, fp, fpm1, rl) awaiting the wv mult

            def emit_wv(item):
                c, ls, LHC, fp, fpm1, rl = item
                wv = gates_pool.tile([P, LHC], F16, tag="wv")
                nc.vector.tensor_tensor(wv[:], fpm1[:], rl[:], Alu.mult)
                for off in range(0, LHC, SSPLIT):
                    Lsub = min(SSPLIT, LHC - off)
                    pend_scans.append((c, ls + off, Lsub, fp, off, wv))

            def emit_subscan(item):
                c, ls, Lsub, fp, off, wv = item
                hv = hvs[c]
                init = HX_INIT if ls == 0 else hv[:, ls - 1 : ls]
                nc.vector.tensor_tensor_scan(
                    hv[:, ls : ls + Lsub], fp[:, off : off + Lsub],
                    wv[:, off : off + Lsub], init,
                    Alu.mult, Alu.subtract,
                )
                nc.sync.dma_start(
                    out_d[c * P : (c + 1) * P, ls : ls + Lsub],
                    hv[:, ls : ls + Lsub],
                )

            lsoff = {c: 0 for c in range(NCT)}
            for c, j in order:
                LHC = CLIST[j]
                ls = lsoff[c]

                sf = gates_pool.tile([P, LHC], F16, tag="sf")
                si = gates_pool.tile([P, LHC], F16, tag="si")
                sh = gates_pool.tile([P, LHC], F16, tag="sh")
                fp = gates_pool.tile([P, LHC], F16, tag="fp")

                def mms8(ps, ocol):
                    for j2 in range(LHC // LT):
                        xk = slice(ls + j2 * LT, ls + (j2 + 1) * LT)
                        jl = slice(j2 * LT, (j2 + 1) * LT)
                        for kp in range(NK // 2):
                            nc.tensor.matmul(
                                ps[:, jl],
                                w8_sb[:, 2 * kp : 2 * kp + 2, ocol : ocol + P],
                                xT8_sb[:, 2 * kp : 2 * kp + 2, xk],
                                start=kp == 0,
                                stop=kp == NK // 2 - 1,
                                perf_mode=DR,
                            )

                def mmsbf(ps, ocol):
                    for j2 in range(LHC // LT):
                        xk = slice(ls + j2 * LT, ls + (j2 + 1) * LT)
                        jl = slice(j2 * LT, (j2 + 1) * LT)
                        for ki in range(NK):
                            nc.tensor.matmul(
                                ps[:, jl],
                                wbf_sb[:, ki, ocol : ocol + P],
                                xTbf_sb[:, ki, xk],
                                start=ki == 0,
                                stop=ki == NK - 1,
                            )

                # F gate
                psF = psum_pool.tile([P, LHC], F32, tag="ps")
                mms8(psF, c * CW8)
                nc.scalar.activation(
                    sf[:], psF[:], Act.Sigmoid,
                    bias=bias_sb[:, 0 * NCT + c : 0 * NCT + c + 1],
                    scale=1.0 / (XS * WS),
                )
                # I gate
                psI = psum_pool.tile([P, LHC], F32, tag="ps")
                mms8(psI, c * CW8 + P)
                nc.scalar.activation(
                    si[:], psI[:], Act.Sigmoid,
                    bias=bias_sb[:, 1 * NCT + c : 1 * NCT + c + 1],
                    scale=1.0 / (XS * WS),
                )
                # H gate
                psH = psum_pool.tile([P, LHC], F32, tag="ps")
                mmsbf(psH, c * P)
                nc.scalar.activation(
                    sh[:], psH[:], Act.Sigmoid,
                    bias=bias_sb[:, 2 * NCT + c : 2 * NCT + c + 1],
                )
                rl = gates_pool.tile([P, LHC], F16, tag="rl")
                nc.scalar.activation(
                    rl[:], psH[:], Act.Relu,
                    bias=bias_sb[:, 2 * NCT + c : 2 * NCT + c + 1],
                )
                # f' = sf/(sf+si), one fused DVE op over the whole chunk
                nc.vector._custom_dve(
                    fract_op, out=fp[:], in0=sf[:], in1=si[:],
                    s0=FR_C0, s1=FR_C1,
                )
                # htil = relu(psH+b_h+0.5) + min(sigmoid(psH+b_h), 0.5):
                # exact g identity; min on DVE (4x mode), add via SWDGE
                # accumulate DMA in-place into rl (off-engine).
                mn = gates_pool.tile([P, LHC], F16, tag="mn")
                nc.vector.tensor_scalar_min(mn[:], sh[:], 0.5)
                nc.gpsimd.dma_start(out=rl[:], in_=mn[:], accum_op=Alu.add)
                fpm1 = gates_pool.tile([P, LHC], F16, tag="fpm1")
                nc.vector.tensor_scalar_add(fpm1[:], fp[:], -1.0)

                # Emit chunk k-2's sub-scans, then chunk k-1's wv multiply
                # (the SWDGE-add round trip needs a slot of slack before the
                # DVE mult reads rl).
                while len(pend_scans) > 4:
                    emit_subscan(pend_scans.pop(0))
                pend_wv.append((c, ls, LHC, fp, fpm1, rl))
                if len(pend_wv) > 1:
                    emit_wv(pend_wv.pop(0))
                lsoff[c] += LHC

            while pend_wv:
                emit_wv(pend_wv.pop(0))
            while pend_scans:
                emit_subscan(pend_scans.pop(0))

    nc.compile()
    _cached_nc[key] = nc
    return nc


def reorder_w8(W: np.ndarray) -> np.ndarray:
    """fp8 weights, [H_IN, NCT*2P] with per-c-tile [F_c | I_c] col groups."""
    Wf, Wi = W[:H], W[H : 2 * H]
    cols = []
    for c in range(NCT):
        cols.append(Wf[c * P : (c + 1) * P])
        cols.append(Wi[c * P : (c + 1) * P])
    w = np.concatenate(cols, axis=0)  # [NCT*2P, H_IN]
    w8 = np.clip(w.T * WS, -240.0, 240.0).astype(ml_dtypes.float8_e4m3fn)
    return np.ascontiguousarray(w8)


def reorder_w16(W: np.ndarray) -> np.ndarray:
    Wh = W[2 * H :]
    return np.ascontiguousarray(Wh.T.astype(ml_dtypes.bfloat16))


def make_bias(b: np.ndarray) -> np.ndarray:
    b32 = np.asarray(b, dtype=np.float32)
    bias = np.empty((P, 16), dtype=np.float32)
    for c in range(NCT):
        bias[:, 0 * NCT + c] = b32[0 * H + c * P : 0 * H + (c + 1) * P]
        bias[:, 1 * NCT + c] = b32[1 * H + c * P : 1 * H + (c + 1) * P]
        bias[:, 2 * NCT + c] = b32[2 * H + c * P : 2 * H + (c + 1) * P]
        bias[:, 3 * NCT + c] = b32[2 * H + c * P : 2 * H + (c + 1) * P] + 0.5
    return bias


def _prep_core_inputs(x_n, wbf, w8, bias):
    xT = np.ascontiguousarray(x_n.T)
    return {
        "xTbf": xT.astype(ml_dtypes.bfloat16),
        "xT8": np.clip(xT * XS, -240.0, 240.0).astype(ml_dtypes.float8_e4m3fn),
        "wbf": wbf,
        "w8": w8,
        "bias": bias,
    }


def kernel(x: np.ndarray, W: np.ndarray, b: np.ndarray) -> np.ndarray:
    from concourse.bass_utils import run_bass_kernel_spmd

    nc = build_program()

    W = np.asarray(W)
    wbf = reorder_w16(W)
    w8 = reorder_w8(W)
    bias = make_bias(b)

    in_maps = [_prep_core_inputs(np.asarray(x[n]), wbf, w8, bias) for n in range(N)]
    res = run_bass_kernel_spmd(nc, in_maps, list(range(N)))

    out = np.empty((N, L, H), dtype=np.float32)
    for n in range(N):
        out[n] = res.results[n]["out"].T.astype(np.float32)
    return out


# revision 10
# speedup vs baseline: 1.0324x; 1.0000x over previous
"""MinLSTM cell (Heinsen-scan reference) as a Bass/Tile kernel for 8 trn2 NeuronCores.

Linear-space rewrite of the reference's log-space scan:
    h_t = f'_t h_{t-1} + (1 - f'_t) g(pre_h_t),   h_0 = 1e-6
with f' = sigmoid(pre_f+b_f) / (sigmoid(pre_f+b_f) + sigmoid(pre_i+b_i)) and
g(x) = max(x+0.5, sigmoid(x)).

Distribution: data-parallel over batch N=8, one batch element per core, W/b
replicated. Device layout: channels on SBUF partitions (4 c-tiles of 128),
sequence along the free dim. Chunks are emitted round-robin across c-tiles
so each c-tile's sequential scan chain has three other chunks of work
between dependent scans.

Per-core engine assignment:
  PE : F/I gate matmuls in fp8 E4M3 (x*16, W*64) with DoubleRow perf mode,
       H gate matmul in bf16 (fp16 measures 427ns/512col on HW vs bf16's
       ~216 - fp16 runs at half rate).
  ACT: sf = sigmoid(psF/1024 + b_f); si = sigmoid(psI/1024 + b_i);
       sh = sigmoid(psH + b_h); rl = relu(psH + b_h + 0.5). PSUM is drained
       by ACT alone so the psum rotation is a pure PE<->ACT loop.
  DVE: fp = FRACT_FAST_ANT(sf, si) = sf/(sf+si) fused custom op;
       htl = max(rl, sh) (2x-mode tensor_tensor; exact g identity
       g(x) = max(relu(x+0.5), sigmoid(x)));
       scans h = tensor_tensor_scan(fp, wv, mult, subtract) as 2 sub-scans
       per chunk, deferred 1-2 slots for pipeline depth.
  DVE also: fpm1 = fp - 1 (tensor_scalar, 4x mode); then wv is formed by
       a SWDGE accumulate-mult DMA (fpm1 *= htl) so the multiply runs on
       the DMA engines, off every compute engine. GPSIMD tensor_tensor is
       deliberately unused: concurrent Pool-engine SBUF traffic inflates
       the DVE scan's loop-carried latency ~2.6x (measured).
  GPS: SWDGE descriptor generation only.
  SP : all HBM loads/stores.
"""

import os
import sys

import numpy as np

sys.path.insert(0, "/opt/trn_rl_repo")

import ml_dtypes  # noqa: E402

import concourse.bass as bass  # noqa: E402
import concourse.tile as tile  # noqa: E402
from concourse import bacc, mybir  # noqa: E402
from concourse import dve_ops  # noqa: E402
from concourse.dve_spec import (  # noqa: E402
    AluOp,
    Bin,
    C0,
    C1,
    Spec,
    lower,
)
from concourse.dve_uop import DveOpSpec  # noqa: E402

N, L, H_IN, H = 8, 4096, 512, 512
P = 128
NK = H_IN // P  # 4 k-blocks of the contraction dim
NCT = H // P  # 4 channel tiles
LT = 512  # matmul moving tile (one PSUM bank of fp32)
F32 = mybir.dt.float32
F16 = mybir.dt.float16
BF16 = mybir.dt.bfloat16
F8 = mybir.dt.float8e4
Alu = mybir.AluOpType
Act = mybir.ActivationFunctionType
DR = mybir.MatmulPerfMode.DoubleRow

HX_INIT = 1e-6
XS, WS = 16.0, 64.0  # fp8 scale for x and W (TRN E4M3 max is +-240)
FR_C0, FR_C1 = -0.23549792, 2.0017324  # recip bit-seed Chebyshev consts

# chunk column-lists per c-tile: small first round so the pipeline
# starts as soon as the first 128KB of x8 lands
CLIST = [512, 2048, 1536]
# sub-scan split within a chunk (pipeline depth without extra ACT instrs)
SSPLIT = 1024

_cached_nc = {}
_fract_op = None


def _register_dve_ops():
    """Author + register the fused f' = sf/(sf+si) DVE op (bit-trick recip
    seed + one Newton step + multiply, ~0.17% max rel err)."""
    global _fract_op
    if _fract_op is not None:
        return _fract_op

    def _np_recip_seed_nr1(s, c0, c1):
        ns = (~s.view(np.int32)).view(np.float32)
        y0 = ns * c0
        return y0 * (c1 - s * y0)

    def _ref_fract(in0, in1, c0, c1, c2):
        sf = in0.astype(np.float32)
        s = sf + in1.astype(np.float32)
        return sf * _np_recip_seed_nr1(s, c0, c1)

    from concourse.dve_spec import Src0, Src1  # noqa: E402

    s_expr = Src0 + Src1
    not_s = Bin(AluOp.BITWISE_NOT, s_expr, s_expr)
    y0 = not_s * C0
    y1 = y0 * (C1 - s_expr * y0)
    fract_spec = Spec(body=Src0 * y1, reference=_ref_fract)

    name = "FRACT_FAST_ANT"
    existing = next((o for o in dve_ops.OPS if o.name == name), None)
    if existing is not None:
        _fract_op = existing
        return _fract_op
    row = dve_ops._CUSTOM_DVE_ROW_BASE + len(dve_ops.OPS)
    shas = {}
    for ver in ("v3",):
        tmp = DveOpSpec(
            name=name,
            opcode=row,
            uops=lower(fract_spec, ver=ver),
            rd1_en=True,
        )
        shas[ver] = tmp.sha(ver)
    op = dve_ops.DveOp(name=name, spec=fract_spec, subdim=False, uops_sha=shas)
    dve_ops.OPS.append(op)
    dve_ops._SUB_OPCODE_FOR_NAME[name] = row
    dve_ops.CUSTOM_DVE_SPECS[name] = fract_spec
    _fract_op = op
    return _fract_op


def build_program():
    key = 0
    if key in _cached_nc:
        return _cached_nc[key]
    fract_op = _register_dve_ops()

    nc = bacc.Bacc()
    xTbf_d = nc.dram_tensor("xTbf", [H_IN, L], BF16, kind="ExternalInput")
    xT8_d = nc.dram_tensor("xT8", [H_IN, L], F8, kind="ExternalInput")
    wbf_d = nc.dram_tensor("wbf", [H_IN, NCT * P], BF16, kind="ExternalInput")
    w8_d = nc.dram_tensor("w8", [H_IN, NCT * 2 * P], F8, kind="ExternalInput")
    bias_d = nc.dram_tensor("bias", [P, 16], F32, kind="ExternalInput")
    out_d = nc.dram_tensor("out", [H, L], F16, kind="ExternalOutput")

    CW8 = 2 * P  # fp8 weight cols per c-tile: [F_c | I_c]
    LMAX = max(CLIST)

    with tile.TileContext(nc) as tc:
        with (
            tc.tile_pool(name="const", bufs=1) as const_pool,
            tc.tile_pool(name="gates", bufs=3) as gates_pool,
            tc.tile_pool(name="scanbuf", bufs=1) as scan_pool,
            tc.tile_pool(name="psum", bufs=2, space="PSUM") as psum_pool,
        ):
            # Warmup activation: absorbs the one-time sigmoid act-table load.
            warm = const_pool.tile([P, 8], F32)
            nc.vector.memset(warm[:], 0.0)
            nc.scalar.activation(warm[:], warm[:], Act.Sigmoid)
            # PE warmup: garbage matmuls with no deps so the HAM clock gate
            # reaches 2.4GHz while the first DMAs are in flight.
            wup = const_pool.tile([P, P], BF16)
            nc.vector.memset(wup[:], 0.0)
            wup_ps = psum_pool.tile([P, P], F32, tag="ps")
            for _ in range(12):
                nc.tensor.matmul(wup_ps[:], wup[:], wup[:], start=True, stop=True)

            xTbf_sb = const_pool.tile([P, NK, L], BF16)
            xT8_sb = const_pool.tile([P, NK, L], F8)
            wbf_sb = const_pool.tile([P, NK, NCT * P], BF16)
            w8_sb = const_pool.tile([P, NK, NCT * CW8], F8)
            bias_sb = const_pool.tile([P, 16], F32)

            w8_r = w8_d.rearrange("(ki p) o -> p ki o", p=P)
            wbf_r = wbf_d.rearrange("(ki p) o -> p ki o", p=P)
            xTbf_r = xTbf_d.rearrange("(ki p) l -> p ki l", p=P)
            xT8_r = xT8_d.rearrange("(ki p) l -> p ki l", p=P)

            # Load order prioritizes the first round-robin sweep (j=0, 1024
            # cols, all four c-tiles): w8[c0], x8 head, wbf[c0], bias, xbf
            # head, the other c-tiles' weights, then the remaining x chunks.
            xsplits = []
            off = 0
            for sz in CLIST:
                xsplits.append((off, off + sz))
                off += sz

            nc.sync.dma_start(w8_sb[:, :, 0:CW8], w8_r[:, :, 0:CW8])
            s0, e0 = xsplits[0]
            # First-chunk x loads in 512-col pieces: the first 512-col matmul
            # group only needs the first piece, so PE starts ~6us in instead
            # of waiting for the full first chunk.
            nc.sync.dma_start(xT8_sb[:, :, s0 : s0 + 512], xT8_r[:, :, s0 : s0 + 512])
            nc.sync.dma_start(wbf_sb[:, :, 0:P], wbf_r[:, :, 0:P])
            nc.sync.dma_start(bias_sb[:], bias_d[:])
            nc.sync.dma_start(xTbf_sb[:, :, s0 : s0 + 512], xTbf_r[:, :, s0 : s0 + 512])
            for p in range(s0 + 512, e0, 512):
                nc.sync.dma_start(xT8_sb[:, :, p : p + 512], xT8_r[:, :, p : p + 512])
                nc.sync.dma_start(xTbf_sb[:, :, p : p + 512], xTbf_r[:, :, p : p + 512])
            for cg in range(1, NCT):
                nc.sync.dma_start(
                    w8_sb[:, :, cg * CW8 : (cg + 1) * CW8],
                    w8_r[:, :, cg * CW8 : (cg + 1) * CW8],
                )
                nc.sync.dma_start(
                    wbf_sb[:, :, cg * P : (cg + 1) * P],
                    wbf_r[:, :, cg * P : (cg + 1) * P],
                )
            for s, e in xsplits[1:]:
                nc.sync.dma_start(xT8_sb[:, :, s:e], xT8_r[:, :, s:e])
                nc.sync.dma_start(xTbf_sb[:, :, s:e], xTbf_r[:, :, s:e])

            hvs = {
                c: scan_pool.tile([P, L], F16, tag=f"hv{c}", name=f"hv{c}")
                for c in range(NCT)
            }

            order = [(c, j) for j in range(len(CLIST)) for c in range(NCT)]

            # Sub-scan deferral: chunk k's first sub-scan is emitted with
            # chunk k+1's head, its second with chunk k+2's head. The GPS
            # fpm1/wv pair is emitted inline (lag-0) - the GPS queue itself
            # provides buffering since nothing else runs there.
            pend_scans = []  # flat list of (c, ls, Lsub, fp, fp_off, wv)
            pend_wv = []  # (c, ls, LHC, fp, fpm1, rl) awaiting the wv mult

            def emit_wv(item):
                c, ls, LHC, fp, fpm1, rl = item
                wv = gates_pool.tile([P, LHC], F16, tag="wv")
                nc.vector.tensor_tensor(wv[:], fpm1[:], rl[:], Alu.mult)
                for off in range(0, LHC, SSPLIT):
                    Lsub = min(SSPLIT, LHC - off)
                    pend_scans.append((c, ls + off, Lsub, fp, off, wv))

            def emit_subscan(item):
                c, ls, Lsub, fp, off, wv = item
                hv = hvs[c]
                init = HX_INIT if ls == 0 else hv[:, ls - 1 : ls]
                nc.vector.tensor_tensor_scan(
                    hv[:, ls : ls + Lsub], fp[:, off : off + Lsub],
                    wv[:, off : off + Lsub], init,
                    Alu.mult, Alu.subtract,
                )
                nc.sync.dma_start(
                    out_d[c * P : (c + 1) * P, ls : ls + Lsub],
                    hv[:, ls : ls + Lsub],
                )

            lsoff = {c: 0 for c in range(NCT)}
            for c, j in order:
                LHC = CLIST[j]
                ls = lsoff[c]

                sf = gates_pool.tile([P, LHC], F16, tag="sf")
                si = gates_pool.tile([P, LHC], F16, tag="si")
                sh = gates_pool.tile([P, LHC], F16, tag="sh")
                fp = gates_pool.tile([P, LHC], F16, tag="fp")

                def mms8(ps, ocol):
                    for j2 in range(LHC // LT):
                        xk = slice(ls + j2 * LT, ls + (j2 + 1) * LT)
                        jl = slice(j2 * LT, (j2 + 1) * LT)
                        for kp in range(NK // 2):
                            nc.tensor.matmul(
                                ps[:, jl],
                                w8_sb[:, 2 * kp : 2 * kp + 2, ocol : ocol + P],
                                xT8_sb[:, 2 * kp : 2 * kp + 2, xk],
                                start=kp == 0,
                                stop=kp == NK // 2 - 1,
                                perf_mode=DR,
                            )

                def mmsbf(ps, ocol):
                    for j2 in range(LHC // LT):
                        xk = slice(ls + j2 * LT, ls + (j2 + 1) * LT)
                        jl = slice(j2 * LT, (j2 + 1) * LT)
                        for ki in range(NK):
                            nc.tensor.matmul(
                                ps[:, jl],
                                wbf_sb[:, ki, ocol : ocol + P],
                                xTbf_sb[:, ki, xk],
                                start=ki == 0,
                                stop=ki == NK - 1,
                            )

                # F gate
                psF = psum_pool.tile([P, LHC], F32, tag="ps")
                mms8(psF, c * CW8)
                nc.scalar.activation(
                    sf[:], psF[:], Act.Sigmoid,
                    bias=bias_sb[:, 0 * NCT + c : 0 * NCT + c + 1],
                    scale=1.0 / (XS * WS),
                )
                # I gate
                psI = psum_pool.tile([P, LHC], F32, tag="ps")
                mms8(psI, c * CW8 + P)
                nc.scalar.activation(
                    si[:], psI[:], Act.Sigmoid,
                    bias=bias_sb[:, 1 * NCT + c : 1 * NCT + c + 1],
                    scale=1.0 / (XS * WS),
                )
                # H gate
                psH = psum_pool.tile([P, LHC], F32, tag="ps")
                mmsbf(psH, c * P)
                nc.scalar.activation(
                    sh[:], psH[:], Act.Sigmoid,
                    bias=bias_sb[:, 2 * NCT + c : 2 * NCT + c + 1],
                )
                rl = gates_pool.tile([P, LHC], F16, tag="rl")
                nc.scalar.activation(
                    rl[:], psH[:], Act.Relu,
                    bias=bias_sb[:, 2 * NCT + c : 2 * NCT + c + 1],
                )
                # f' = sf/(sf+si), one fused DVE op over the whole chunk
                nc.vector._custom_dve(
                    fract_op, out=fp[:], in0=sf[:], in1=si[:],
                    s0=FR_C0, s1=FR_C1,
                )
                # htil = relu(psH+b_h+0.5) + min(sigmoid(psH+b_h), 0.5):
                # exact g identity; min on DVE (4x mode), add via SWDGE
                # accumulate DMA in-place into rl (off-engine).
                mn = gates_pool.tile([P, LHC], F16, tag="mn")
                nc.vector.tensor_scalar_min(mn[:], sh[:], 0.5)
                nc.gpsimd.dma_start(out=rl[:], in_=mn[:], accum_op=Alu.add)
                fpm1 = gates_pool.tile([P, LHC], F16, tag="fpm1")
                nc.vector.tensor_scalar_add(fpm1[:], fp[:], -1.0)

                # Emit chunk k-2's sub-scans, then chunk k-1's wv multiply
                # (the SWDGE-add round trip needs a slot of slack before the
                # DVE mult reads rl).
                while len(pend_scans) > 4:
                    emit_subscan(pend_scans.pop(0))
                pend_wv.append((c, ls, LHC, fp, fpm1, rl))
                if len(pend_wv) > 2:
                    emit_wv(pend_wv.pop(0))
                lsoff[c] += LHC

            while pend_wv:
                emit_wv(pend_wv.pop(0))
            while pend_scans:
                emit_subscan(pend_scans.pop(0))

    nc.compile()
    _cached_nc[key] = nc
    return nc


def reorder_w8(W: np.ndarray) -> np.ndarray:
    """fp8 weights, [H_IN, NCT*2P] with per-c-tile [F_c | I_c] col groups."""
    Wf, Wi = W[:H], W[H : 2 * H]
    cols = []
    for c in range(NCT):
        cols.append(Wf[c * P : (c + 1) * P])
        cols.append(Wi[c * P : (c + 1) * P])
    w = np.concatenate(cols, axis=0)  # [NCT*2P, H_IN]
    w8 = np.clip(w.T * WS, -240.0, 240.0).astype(ml_dtypes.float8_e4m3fn)
    return np.ascontiguousarray(w8)


def reorder_w16(W: np.ndarray) -> np.ndarray:
    Wh = W[2 * H :]
    return np.ascontiguousarray(Wh.T.astype(ml_dtypes.bfloat16))


def make_bias(b: np.ndarray) -> np.ndarray:
    b32 = np.asarray(b, dtype=np.float32)
    bias = np.empty((P, 16), dtype=np.float32)
    for c in range(NCT):
        bias[:, 0 * NCT + c] = b32[0 * H + c * P : 0 * H + (c + 1) * P]
        bias[:, 1 * NCT + c] = b32[1 * H + c * P : 1 * H + (c + 1) * P]
        bias[:, 2 * NCT + c] = b32[2 * H + c * P : 2 * H + (c + 1) * P]
        bias[:, 3 * NCT + c] = b32[2 * H + c * P : 2 * H + (c + 1) * P] + 0.5
    return bias


def _prep_core_inputs(x_n, wbf, w8, bias):
    xT = np.ascontiguousarray(x_n.T)
    return {
        "xTbf": xT.astype(ml_dtypes.bfloat16),
        "xT8": np.clip(xT * XS, -240.0, 240.0).astype(ml_dtypes.float8_e4m3fn),
        "wbf": wbf,
        "w8": w8,
        "bias": bias,
    }


def kernel(x: np.ndarray, W: np.ndarray, b: np.ndarray) -> np.ndarray:
    from concourse.bass_utils import run_bass_kernel_spmd

    nc = build_program()

    W = np.asarray(W)
    wbf = reorder_w16(W)
    w8 = reorder_w8(W)
    bias = make_bias(b)

    in_maps = [_prep_core_inputs(np.asarray(x[n]), wbf, w8, bias) for n in range(N)]
    res = run_bass_kernel_spmd(nc, in_maps, list(range(N)))

    out = np.empty((N, L, H), dtype=np.float32)
    for n in range(N):
        out[n] = res.results[n]["out"].T.astype(np.float32)
    return out


# revision 11
# speedup vs baseline: 1.0349x; 1.0024x over previous
"""MinLSTM cell (Heinsen-scan reference) as a Bass/Tile kernel for 8 trn2 NeuronCores.

Linear-space rewrite of the reference's log-space scan:
    h_t = f'_t h_{t-1} + (1 - f'_t) g(pre_h_t),   h_0 = 1e-6
with f' = sigmoid(pre_f+b_f) / (sigmoid(pre_f+b_f) + sigmoid(pre_i+b_i)) and
g(x) = max(x+0.5, sigmoid(x)).

Distribution: data-parallel over batch N=8, one batch element per core, W/b
replicated. Device layout: channels on SBUF partitions (4 c-tiles of 128),
sequence along the free dim. Chunks are emitted round-robin across c-tiles
so each c-tile's sequential scan chain has three other chunks of work
between dependent scans.

Per-core engine assignment:
  PE : F/I gate matmuls in fp8 E4M3 (x*16, W*64) with DoubleRow perf mode,
       H gate matmul in bf16 (fp16 measures 427ns/512col on HW vs bf16's
       ~216 - fp16 runs at half rate).
  ACT: sf = sigmoid(psF/1024 + b_f); si = sigmoid(psI/1024 + b_i);
       sh = sigmoid(psH + b_h); rl = relu(psH + b_h + 0.5). PSUM is drained
       by ACT alone so the psum rotation is a pure PE<->ACT loop.
  DVE: fp = FRACT_FAST_ANT(sf, si) = sf/(sf+si) fused custom op;
       htl = max(rl, sh) (2x-mode tensor_tensor; exact g identity
       g(x) = max(relu(x+0.5), sigmoid(x)));
       scans h = tensor_tensor_scan(fp, wv, mult, subtract) as 2 sub-scans
       per chunk, deferred 1-2 slots for pipeline depth.
  DVE also: fpm1 = fp - 1 (tensor_scalar, 4x mode); then wv is formed by
       a SWDGE accumulate-mult DMA (fpm1 *= htl) so the multiply runs on
       the DMA engines, off every compute engine. GPSIMD tensor_tensor is
       deliberately unused: concurrent Pool-engine SBUF traffic inflates
       the DVE scan's loop-carried latency ~2.6x (measured).
  GPS: SWDGE descriptor generation only.
  SP : all HBM loads/stores.
"""

import os
import sys

import numpy as np

sys.path.insert(0, "/opt/trn_rl_repo")

import ml_dtypes  # noqa: E402

import concourse.bass as bass  # noqa: E402
import concourse.tile as tile  # noqa: E402
from concourse import bacc, mybir  # noqa: E402
from concourse import dve_ops  # noqa: E402
from concourse.dve_spec import (  # noqa: E402
    AluOp,
    Bin,
    C0,
    C1,
    Spec,
    lower,
)
from concourse.dve_uop import DveOpSpec  # noqa: E402

N, L, H_IN, H = 8, 4096, 512, 512
P = 128
NK = H_IN // P  # 4 k-blocks of the contraction dim
NCT = H // P  # 4 channel tiles
LT = 512  # matmul moving tile (one PSUM bank of fp32)
F32 = mybir.dt.float32
F16 = mybir.dt.float16
BF16 = mybir.dt.bfloat16
F8 = mybir.dt.float8e4
Alu = mybir.AluOpType
Act = mybir.ActivationFunctionType
DR = mybir.MatmulPerfMode.DoubleRow

HX_INIT = 1e-6
XS, WS = 16.0, 64.0  # fp8 scale for x and W (TRN E4M3 max is +-240)
FR_C0, FR_C1 = -0.23549792, 2.0017324  # recip bit-seed Chebyshev consts

# chunk column-lists per c-tile: small first round so the pipeline
# starts as soon as the first 128KB of x8 lands
CLIST = [512, 2048, 1536]
# sub-scan split within a chunk (pipeline depth without extra ACT instrs)
SSPLIT = 1024

_cached_nc = {}
_fract_op = None


def _register_dve_ops():
    """Author + register the fused f' = sf/(sf+si) DVE op (bit-trick recip
    seed + one Newton step + multiply, ~0.17% max rel err)."""
    global _fract_op
    if _fract_op is not None:
        return _fract_op

    def _np_recip_seed_nr1(s, c0, c1):
        ns = (~s.view(np.int32)).view(np.float32)
        y0 = ns * c0
        return y0 * (c1 - s * y0)

    def _ref_fract(in0, in1, c0, c1, c2):
        sf = in0.astype(np.float32)
        s = sf + in1.astype(np.float32)
        return sf * _np_recip_seed_nr1(s, c0, c1)

    from concourse.dve_spec import Src0, Src1  # noqa: E402

    s_expr = Src0 + Src1
    not_s = Bin(AluOp.BITWISE_NOT, s_expr, s_expr)
    y0 = not_s * C0
    y1 = y0 * (C1 - s_expr * y0)
    fract_spec = Spec(body=Src0 * y1, reference=_ref_fract)

    name = "FRACT_FAST_ANT"
    existing = next((o for o in dve_ops.OPS if o.name == name), None)
    if existing is not None:
        _fract_op = existing
        return _fract_op
    row = dve_ops._CUSTOM_DVE_ROW_BASE + len(dve_ops.OPS)
    shas = {}
    for ver in ("v3",):
        tmp = DveOpSpec(
            name=name,
            opcode=row,
            uops=lower(fract_spec, ver=ver),
            rd1_en=True,
        )
        shas[ver] = tmp.sha(ver)
    op = dve_ops.DveOp(name=name, spec=fract_spec, subdim=False, uops_sha=shas)
    dve_ops.OPS.append(op)
    dve_ops._SUB_OPCODE_FOR_NAME[name] = row
    dve_ops.CUSTOM_DVE_SPECS[name] = fract_spec
    _fract_op = op
    return _fract_op


def build_program():
    key = 0
    if key in _cached_nc:
        return _cached_nc[key]
    fract_op = _register_dve_ops()

    nc = bacc.Bacc()
    xTbf_d = nc.dram_tensor("xTbf", [H_IN, L], BF16, kind="ExternalInput")
    xT8_d = nc.dram_tensor("xT8", [H_IN, L], F8, kind="ExternalInput")
    wbf_d = nc.dram_tensor("wbf", [H_IN, NCT * P], BF16, kind="ExternalInput")
    w8_d = nc.dram_tensor("w8", [H_IN, NCT * 2 * P], F8, kind="ExternalInput")
    bias_d = nc.dram_tensor("bias", [P, 16], F32, kind="ExternalInput")
    out_d = nc.dram_tensor("out", [H, L], F16, kind="ExternalOutput")

    CW8 = 2 * P  # fp8 weight cols per c-tile: [F_c | I_c]
    LMAX = max(CLIST)

    with tile.TileContext(nc) as tc:
        with (
            tc.tile_pool(name="const", bufs=1) as const_pool,
            tc.tile_pool(name="gates", bufs=3) as gates_pool,
            tc.tile_pool(name="scanbuf", bufs=1) as scan_pool,
            tc.tile_pool(name="psum", bufs=2, space="PSUM") as psum_pool,
        ):
            # Warmup activation: absorbs the one-time sigmoid act-table load.
            warm = const_pool.tile([P, 8], F32)
            nc.vector.memset(warm[:], 0.0)
            nc.scalar.activation(warm[:], warm[:], Act.Sigmoid)
            # PE warmup: garbage matmuls with no deps so the HAM clock gate
            # reaches 2.4GHz while the first DMAs are in flight.
            wup = const_pool.tile([P, P], BF16)
            nc.vector.memset(wup[:], 0.0)
            wup_ps = psum_pool.tile([P, P], F32, tag="ps")
            for _ in range(12):
                nc.tensor.matmul(wup_ps[:], wup[:], wup[:], start=True, stop=True)

            xTbf_sb = const_pool.tile([P, NK, L], BF16)
            xT8_sb = const_pool.tile([P, NK, L], F8)
            wbf_sb = const_pool.tile([P, NK, NCT * P], BF16)
            w8_sb = const_pool.tile([P, NK, NCT * CW8], F8)
            bias_sb = const_pool.tile([P, 16], F32)

            w8_r = w8_d.rearrange("(ki p) o -> p ki o", p=P)
            wbf_r = wbf_d.rearrange("(ki p) o -> p ki o", p=P)
            xTbf_r = xTbf_d.rearrange("(ki p) l -> p ki l", p=P)
            xT8_r = xT8_d.rearrange("(ki p) l -> p ki l", p=P)

            # Load order prioritizes the first round-robin sweep (j=0, 1024
            # cols, all four c-tiles): w8[c0], x8 head, wbf[c0], bias, xbf
            # head, the other c-tiles' weights, then the remaining x chunks.
            xsplits = []
            off = 0
            for sz in CLIST:
                xsplits.append((off, off + sz))
                off += sz

            nc.sync.dma_start(w8_sb[:, :, 0:CW8], w8_r[:, :, 0:CW8])
            s0, e0 = xsplits[0]
            # First-chunk x loads in 512-col pieces: the first 512-col matmul
            # group only needs the first piece, so PE starts ~6us in instead
            # of waiting for the full first chunk.
            nc.sync.dma_start(xT8_sb[:, :, s0 : s0 + 512], xT8_r[:, :, s0 : s0 + 512])
            nc.sync.dma_start(wbf_sb[:, :, 0:P], wbf_r[:, :, 0:P])
            nc.sync.dma_start(bias_sb[:], bias_d[:])
            nc.sync.dma_start(xTbf_sb[:, :, s0 : s0 + 512], xTbf_r[:, :, s0 : s0 + 512])
            for p in range(s0 + 512, e0, 512):
                nc.sync.dma_start(xT8_sb[:, :, p : p + 512], xT8_r[:, :, p : p + 512])
                nc.sync.dma_start(xTbf_sb[:, :, p : p + 512], xTbf_r[:, :, p : p + 512])
            for cg in range(1, NCT):
                nc.sync.dma_start(
                    w8_sb[:, :, cg * CW8 : (cg + 1) * CW8],
                    w8_r[:, :, cg * CW8 : (cg + 1) * CW8],
                )
                nc.sync.dma_start(
                    wbf_sb[:, :, cg * P : (cg + 1) * P],
                    wbf_r[:, :, cg * P : (cg + 1) * P],
                )
            for s, e in xsplits[1:]:
                nc.sync.dma_start(xT8_sb[:, :, s:e], xT8_r[:, :, s:e])
                nc.sync.dma_start(xTbf_sb[:, :, s:e], xTbf_r[:, :, s:e])

            hvs = {
                c: scan_pool.tile([P, L], F16, tag=f"hv{c}", name=f"hv{c}")
                for c in range(NCT)
            }

            order = [(c, j) for j in range(len(CLIST)) for c in range(NCT)]

            # Sub-scan deferral: chunk k's first sub-scan is emitted with
            # chunk k+1's head, its second with chunk k+2's head. The GPS
            # fpm1/wv pair is emitted inline (lag-0) - the GPS queue itself
            # provides buffering since nothing else runs there.
            pend_scans = []  # flat list of (c, ls, Lsub, fp, fp_off, wv)
            pend_wv = []  # (c, ls, LHC, fp, fpm1, rl) awaiting the wv mult

            def emit_wv(item):
                c, ls, LHC, fp, fpm1, rl = item
                wv = gates_pool.tile([P, LHC], F16, tag="wv")
                nc.vector.tensor_tensor(wv[:], fpm1[:], rl[:], Alu.mult)
                for off in range(0, LHC, SSPLIT):
                    Lsub = min(SSPLIT, LHC - off)
                    pend_scans.append((c, ls + off, Lsub, fp, off, wv))

            def emit_subscan(item):
                c, ls, Lsub, fp, off, wv = item
                hv = hvs[c]
                init = HX_INIT if ls == 0 else hv[:, ls - 1 : ls]
                nc.vector.tensor_tensor_scan(
                    hv[:, ls : ls + Lsub], fp[:, off : off + Lsub],
                    wv[:, off : off + Lsub], init,
                    Alu.mult, Alu.subtract,
                )
                nc.sync.dma_start(
                    out_d[c * P : (c + 1) * P, ls : ls + Lsub],
                    hv[:, ls : ls + Lsub],
                )

            lsoff = {c: 0 for c in range(NCT)}
            for c, j in order:
                LHC = CLIST[j]
                ls = lsoff[c]

                sf = gates_pool.tile([P, LHC], F16, tag="sf")
                si = gates_pool.tile([P, LHC], F16, tag="si")
                sh = gates_pool.tile([P, LHC], F16, tag="sh")
                fp = gates_pool.tile([P, LHC], F16, tag="fp")

                def mms8(ps, ocol):
                    for j2 in range(LHC // LT):
                        xk = slice(ls + j2 * LT, ls + (j2 + 1) * LT)
                        jl = slice(j2 * LT, (j2 + 1) * LT)
                        for kp in range(NK // 2):
                            nc.tensor.matmul(
                                ps[:, jl],
                                w8_sb[:, 2 * kp : 2 * kp + 2, ocol : ocol + P],
                                xT8_sb[:, 2 * kp : 2 * kp + 2, xk],
                                start=kp == 0,
                                stop=kp == NK // 2 - 1,
                                perf_mode=DR,
                            )

                def mmsbf(ps, ocol):
                    for j2 in range(LHC // LT):
                        xk = slice(ls + j2 * LT, ls + (j2 + 1) * LT)
                        jl = slice(j2 * LT, (j2 + 1) * LT)
                        for ki in range(NK):
                            nc.tensor.matmul(
                                ps[:, jl],
                                wbf_sb[:, ki, ocol : ocol + P],
                                xTbf_sb[:, ki, xk],
                                start=ki == 0,
                                stop=ki == NK - 1,
                            )

                # F gate
                psF = psum_pool.tile([P, LHC], F32, tag="ps")
                mms8(psF, c * CW8)
                nc.scalar.activation(
                    sf[:], psF[:], Act.Sigmoid,
                    bias=bias_sb[:, 0 * NCT + c : 0 * NCT + c + 1],
                    scale=1.0 / (XS * WS),
                )
                # I gate
                psI = psum_pool.tile([P, LHC], F32, tag="ps")
                mms8(psI, c * CW8 + P)
                nc.scalar.activation(
                    si[:], psI[:], Act.Sigmoid,
                    bias=bias_sb[:, 1 * NCT + c : 1 * NCT + c + 1],
                    scale=1.0 / (XS * WS),
                )
                # H gate
                psH = psum_pool.tile([P, LHC], F32, tag="ps")
                mmsbf(psH, c * P)
                nc.scalar.activation(
                    sh[:], psH[:], Act.Sigmoid,
                    bias=bias_sb[:, 2 * NCT + c : 2 * NCT + c + 1],
                )
                # Route selection: first round (small chunks) combines g on
                # DVE directly (max-route, relu bias b_h+0.5) - the SWDGE
                # round-trip latency would dominate the short fill-phase
                # slots. Steady-state big chunks use the min+SWDGE-add route
                # (relu bias b_h) to keep DVE lean.
                swdge_route = j > 0
                bcol = (2 if swdge_route else 3) * NCT + c
                rl = gates_pool.tile([P, LHC], F16, tag="rl")
                nc.scalar.activation(
                    rl[:], psH[:], Act.Relu,
                    bias=bias_sb[:, bcol : bcol + 1],
                )
                # f' = sf/(sf+si), one fused DVE op over the whole chunk
                nc.vector._custom_dve(
                    fract_op, out=fp[:], in0=sf[:], in1=si[:],
                    s0=FR_C0, s1=FR_C1,
                )
                # htil: min-route = relu(psH+b_h) + min(sigmoid, 0.5) with
                # the add as a SWDGE accumulate DMA; max-route =
                # max(relu(psH+b_h+0.5), sigmoid) in-place on DVE (2x mode).
                if swdge_route:
                    mn = gates_pool.tile([P, LHC], F16, tag="mn")
                    nc.vector.tensor_scalar_min(mn[:], sh[:], 0.5)
                    nc.gpsimd.dma_start(out=rl[:], in_=mn[:], accum_op=Alu.add)
                else:
                    nc.vector.tensor_tensor(rl[:], rl[:], sh[:], Alu.max)
                fpm1 = gates_pool.tile([P, LHC], F16, tag="fpm1")
                nc.vector.tensor_scalar_add(fpm1[:], fp[:], -1.0)

                # Emit chunk k-2's sub-scans, then chunk k-1's wv multiply
                # (the SWDGE-add round trip needs a slot of slack before the
                # DVE mult reads rl).
                while len(pend_scans) > 4:
                    emit_subscan(pend_scans.pop(0))
                pend_wv.append((c, ls, LHC, fp, fpm1, rl))
                if len(pend_wv) > 2:
                    emit_wv(pend_wv.pop(0))
                lsoff[c] += LHC

            while pend_wv:
                emit_wv(pend_wv.pop(0))
            while pend_scans:
                emit_subscan(pend_scans.pop(0))

    nc.compile()
    _cached_nc[key] = nc
    return nc


def reorder_w8(W: np.ndarray) -> np.ndarray:
    """fp8 weights, [H_IN, NCT*2P] with per-c-tile [F_c | I_c] col groups."""
    Wf, Wi = W[:H], W[H : 2 * H]
    cols = []
    for c in range(NCT):
        cols.append(Wf[c * P : (c + 1) * P])
        cols.append(Wi[c * P : (c + 1) * P])
    w = np.concatenate(cols, axis=0)  # [NCT*2P, H_IN]
    w8 = np.clip(w.T * WS, -240.0, 240.0).astype(ml_dtypes.float8_e4m3fn)
    return np.ascontiguousarray(w8)


def reorder_w16(W: np.ndarray) -> np.ndarray:
    Wh = W[2 * H :]
    return np.ascontiguousarray(Wh.T.astype(ml_dtypes.bfloat16))


def make_bias(b: np.ndarray) -> np.ndarray:
    b32 = np.asarray(b, dtype=np.float32)
    bias = np.empty((P, 16), dtype=np.float32)
    for c in range(NCT):
        bias[:, 0 * NCT + c] = b32[0 * H + c * P : 0 * H + (c + 1) * P]
        bias[:, 1 * NCT + c] = b32[1 * H + c * P : 1 * H + (c + 1) * P]
        bias[:, 2 * NCT + c] = b32[2 * H + c * P : 2 * H + (c + 1) * P]
        bias[:, 3 * NCT + c] = b32[2 * H + c * P : 2 * H + (c + 1) * P] + 0.5
    return bias


def _prep_core_inputs(x_n, wbf, w8, bias):
    xT = np.ascontiguousarray(x_n.T)
    return {
        "xTbf": xT.astype(ml_dtypes.bfloat16),
        "xT8": np.clip(xT * XS, -240.0, 240.0).astype(ml_dtypes.float8_e4m3fn),
        "wbf": wbf,
        "w8": w8,
        "bias": bias,
    }


def kernel(x: np.ndarray, W: np.ndarray, b: np.ndarray) -> np.ndarray:
    from concourse.bass_utils import run_bass_kernel_spmd

    nc = build_program()

    W = np.asarray(W)
    wbf = reorder_w16(W)
    w8 = reorder_w8(W)
    bias = make_bias(b)

    in_maps = [_prep_core_inputs(np.asarray(x[n]), wbf, w8, bias) for n in range(N)]
    res = run_bass_kernel_spmd(nc, in_maps, list(range(N)))

    out = np.empty((N, L, H), dtype=np.float32)
    for n in range(N):
        out[n] = res.results[n]["out"].T.astype(np.float32)
    return out


# revision 12
# speedup vs baseline: 1.0533x; 1.0178x over previous
"""MinLSTM cell (Heinsen-scan reference) as a Bass/Tile kernel for 8 trn2 NeuronCores.

Linear-space rewrite of the reference's log-space scan:
    h_t = f'_t h_{t-1} + (1 - f'_t) g(pre_h_t),   h_0 = 1e-6
with f' = sigmoid(pre_f+b_f) / (sigmoid(pre_f+b_f) + sigmoid(pre_i+b_i)) and
g(x) = max(x+0.5, sigmoid(x)).

Distribution: data-parallel over batch N=8, one batch element per core, W/b
replicated. Device layout: channels on SBUF partitions (4 c-tiles of 128),
sequence along the free dim. Chunks are emitted round-robin across c-tiles
so each c-tile's sequential scan chain has three other chunks of work
between dependent scans.

Per-core engine assignment:
  PE : F/I gate matmuls in fp8 E4M3 (x*16, W*64) with DoubleRow perf mode,
       H gate matmul in bf16 (fp16 measures 427ns/512col on HW vs bf16's
       ~216 - fp16 runs at half rate).
  ACT: sf = sigmoid(psF/1024 + b_f); si = sigmoid(psI/1024 + b_i);
       sh = sigmoid(psH + b_h); rl = relu(psH + b_h + 0.5). PSUM is drained
       by ACT alone so the psum rotation is a pure PE<->ACT loop.
  DVE: fp = FRACT_FAST_ANT(sf, si) = sf/(sf+si) fused custom op;
       htl = max(rl, sh) (2x-mode tensor_tensor; exact g identity
       g(x) = max(relu(x+0.5), sigmoid(x)));
       scans h = tensor_tensor_scan(fp, wv, mult, subtract) as 2 sub-scans
       per chunk, deferred 1-2 slots for pipeline depth.
  DVE also: fpm1 = fp - 1 (tensor_scalar, 4x mode); then wv is formed by
       a SWDGE accumulate-mult DMA (fpm1 *= htl) so the multiply runs on
       the DMA engines, off every compute engine. GPSIMD tensor_tensor is
       deliberately unused: concurrent Pool-engine SBUF traffic inflates
       the DVE scan's loop-carried latency ~2.6x (measured).
  GPS: SWDGE descriptor generation only.
  SP : all HBM loads/stores.
"""

import os
import sys

import numpy as np

sys.path.insert(0, "/opt/trn_rl_repo")

import ml_dtypes  # noqa: E402

import concourse.bass as bass  # noqa: E402
import concourse.tile as tile  # noqa: E402
from concourse import bacc, mybir  # noqa: E402
from concourse import dve_ops  # noqa: E402
from concourse.dve_spec import (  # noqa: E402
    AluOp,
    Bin,
    C0,
    C1,
    Spec,
    lower,
)
from concourse.dve_uop import DveOpSpec  # noqa: E402

N, L, H_IN, H = 8, 4096, 512, 512
P = 128
NK = H_IN // P  # 4 k-blocks of the contraction dim
NCT = H // P  # 4 channel tiles
LT = 512  # matmul moving tile (one PSUM bank of fp32)
F32 = mybir.dt.float32
F16 = mybir.dt.float16
BF16 = mybir.dt.bfloat16
F8 = mybir.dt.float8e4
Alu = mybir.AluOpType
Act = mybir.ActivationFunctionType
DR = mybir.MatmulPerfMode.DoubleRow

HX_INIT = 1e-6
XS, WS = 16.0, 64.0  # fp8 scale for x and W (TRN E4M3 max is +-240)
FR_C0, FR_C1 = -0.23549792, 2.0017324  # recip bit-seed Chebyshev consts

# chunk column-lists per c-tile: small first round so the pipeline
# starts as soon as the first 128KB of x8 lands
CLIST = [512, 2048, 1536]
# sub-scan split within a chunk (pipeline depth without extra ACT instrs)
SSPLIT = 1024

_cached_nc = {}
_fract_op = None


def _register_dve_ops():
    """Author + register the fused f' = sf/(sf+si) DVE op (bit-trick recip
    seed + one Newton step + multiply, ~0.17% max rel err)."""
    global _fract_op
    if _fract_op is not None:
        return _fract_op

    def _np_recip_seed_nr1(s, c0, c1):
        ns = (~s.view(np.int32)).view(np.float32)
        y0 = ns * c0
        return y0 * (c1 - s * y0)

    def _ref_fract(in0, in1, c0, c1, c2):
        sf = in0.astype(np.float32)
        s = sf + in1.astype(np.float32)
        return sf * _np_recip_seed_nr1(s, c0, c1)

    from concourse.dve_spec import Src0, Src1  # noqa: E402

    s_expr = Src0 + Src1
    not_s = Bin(AluOp.BITWISE_NOT, s_expr, s_expr)
    y0 = not_s * C0
    y1 = y0 * (C1 - s_expr * y0)
    fract_spec = Spec(body=Src0 * y1, reference=_ref_fract)

    name = "FRACT_FAST_ANT"
    existing = next((o for o in dve_ops.OPS if o.name == name), None)
    if existing is not None:
        _fract_op = existing
        return _fract_op
    row = dve_ops._CUSTOM_DVE_ROW_BASE + len(dve_ops.OPS)
    shas = {}
    for ver in ("v3",):
        tmp = DveOpSpec(
            name=name,
            opcode=row,
            uops=lower(fract_spec, ver=ver),
            rd1_en=True,
        )
        shas[ver] = tmp.sha(ver)
    op = dve_ops.DveOp(name=name, spec=fract_spec, subdim=False, uops_sha=shas)
    dve_ops.OPS.append(op)
    dve_ops._SUB_OPCODE_FOR_NAME[name] = row
    dve_ops.CUSTOM_DVE_SPECS[name] = fract_spec
    _fract_op = op
    return _fract_op


def build_program():
    key = 0
    if key in _cached_nc:
        return _cached_nc[key]
    fract_op = _register_dve_ops()

    nc = bacc.Bacc()
    xTbf_d = nc.dram_tensor("xTbf", [H_IN, L], BF16, kind="ExternalInput")
    xT8_d = nc.dram_tensor("xT8", [H_IN, L], F8, kind="ExternalInput")
    wbf_d = nc.dram_tensor("wbf", [H_IN, NCT * P], BF16, kind="ExternalInput")
    w8_d = nc.dram_tensor("w8", [H_IN, NCT * 2 * P], F8, kind="ExternalInput")
    bias_d = nc.dram_tensor("bias", [P, 16], F32, kind="ExternalInput")
    out_d = nc.dram_tensor("out", [H, L], F16, kind="ExternalOutput")

    CW8 = 2 * P  # fp8 weight cols per c-tile: [F_c | I_c]
    LMAX = max(CLIST)

    with tile.TileContext(nc) as tc:
        with (
            tc.tile_pool(name="const", bufs=1) as const_pool,
            tc.tile_pool(name="gates", bufs=3) as gates_pool,
            tc.tile_pool(name="scanbuf", bufs=1) as scan_pool,
            tc.tile_pool(name="psum", bufs=2, space="PSUM") as psum_pool,
        ):
            # Warmup activation: absorbs the one-time sigmoid act-table load.
            warm = const_pool.tile([P, 8], F32)
            nc.vector.memset(warm[:], 0.0)
            nc.scalar.activation(warm[:], warm[:], Act.Sigmoid)
            # PE warmup: garbage matmuls with no deps so the HAM clock gate
            # reaches 2.4GHz while the first DMAs are in flight.
            wup = const_pool.tile([P, P], BF16)
            nc.vector.memset(wup[:], 0.0)
            wup_ps = psum_pool.tile([P, P], F32, tag="ps")
            for _ in range(12):
                nc.tensor.matmul(wup_ps[:], wup[:], wup[:], start=True, stop=True)

            xTbf_sb = const_pool.tile([P, NK, L], BF16)
            xT8_sb = const_pool.tile([P, NK, L], F8)
            wbf_sb = const_pool.tile([P, NK, NCT * P], BF16)
            w8_sb = const_pool.tile([P, NK, NCT * CW8], F8)
            bias_sb = const_pool.tile([P, 16], F32)

            w8_r = w8_d.rearrange("(ki p) o -> p ki o", p=P)
            wbf_r = wbf_d.rearrange("(ki p) o -> p ki o", p=P)
            xTbf_r = xTbf_d.rearrange("(ki p) l -> p ki l", p=P)
            xT8_r = xT8_d.rearrange("(ki p) l -> p ki l", p=P)

            # Load order prioritizes the first round-robin sweep (j=0, 1024
            # cols, all four c-tiles): w8[c0], x8 head, wbf[c0], bias, xbf
            # head, the other c-tiles' weights, then the remaining x chunks.
            xsplits = []
            off = 0
            for sz in CLIST:
                xsplits.append((off, off + sz))
                off += sz

            nc.sync.dma_start(w8_sb[:, :, 0:CW8], w8_r[:, :, 0:CW8])
            s0, e0 = xsplits[0]
            # First-chunk x loads in 512-col pieces: the first 512-col matmul
            # group only needs the first piece, so PE starts ~6us in instead
            # of waiting for the full first chunk.
            nc.sync.dma_start(xT8_sb[:, :, s0 : s0 + 512], xT8_r[:, :, s0 : s0 + 512])
            nc.sync.dma_start(wbf_sb[:, :, 0:P], wbf_r[:, :, 0:P])
            nc.sync.dma_start(bias_sb[:], bias_d[:])
            nc.sync.dma_start(xTbf_sb[:, :, s0 : s0 + 512], xTbf_r[:, :, s0 : s0 + 512])
            for p in range(s0 + 512, e0, 512):
                nc.sync.dma_start(xT8_sb[:, :, p : p + 512], xT8_r[:, :, p : p + 512])
                nc.sync.dma_start(xTbf_sb[:, :, p : p + 512], xTbf_r[:, :, p : p + 512])
            for cg in range(1, NCT):
                nc.sync.dma_start(
                    w8_sb[:, :, cg * CW8 : (cg + 1) * CW8],
                    w8_r[:, :, cg * CW8 : (cg + 1) * CW8],
                )
                nc.sync.dma_start(
                    wbf_sb[:, :, cg * P : (cg + 1) * P],
                    wbf_r[:, :, cg * P : (cg + 1) * P],
                )
            for s, e in xsplits[1:]:
                nc.sync.dma_start(xT8_sb[:, :, s:e], xT8_r[:, :, s:e])
                nc.sync.dma_start(xTbf_sb[:, :, s:e], xTbf_r[:, :, s:e])

            hvs = {
                c: scan_pool.tile([P, L], F16, tag=f"hv{c}", name=f"hv{c}")
                for c in range(NCT)
            }

            # Interleaved order: the first big (j=1) chunks start after only
            # two small j=0 chunks, so PE/ACT stay fed through the
            # small-to-big transition instead of draining.
            order = [
                (0, 0), (1, 0), (2, 0), (0, 1), (3, 0), (1, 1),
                (2, 1), (3, 1), (0, 2), (1, 2), (2, 2), (3, 2),
            ]
            assert sorted(order) == sorted(
                (c, j) for j in range(len(CLIST)) for c in range(NCT)
            )

            # Sub-scan deferral: chunk k's first sub-scan is emitted with
            # chunk k+1's head, its second with chunk k+2's head. The GPS
            # fpm1/wv pair is emitted inline (lag-0) - the GPS queue itself
            # provides buffering since nothing else runs there.
            pend_scans = []  # flat list of (c, ls, Lsub, fp, fp_off, wv)
            pend_wv = []  # (c, ls, LHC, fp, fpm1, rl) awaiting the wv mult

            def emit_wv(item):
                c, ls, LHC, fp, fpm1, rl = item
                wv = gates_pool.tile([P, LHC], F16, tag="wv")
                nc.vector.tensor_tensor(wv[:], fpm1[:], rl[:], Alu.mult)
                for off in range(0, LHC, SSPLIT):
                    Lsub = min(SSPLIT, LHC - off)
                    pend_scans.append((c, ls + off, Lsub, fp, off, wv))

            def emit_subscan(item):
                c, ls, Lsub, fp, off, wv = item
                hv = hvs[c]
                init = HX_INIT if ls == 0 else hv[:, ls - 1 : ls]
                nc.vector.tensor_tensor_scan(
                    hv[:, ls : ls + Lsub], fp[:, off : off + Lsub],
                    wv[:, off : off + Lsub], init,
                    Alu.mult, Alu.subtract,
                )
                nc.sync.dma_start(
                    out_d[c * P : (c + 1) * P, ls : ls + Lsub],
                    hv[:, ls : ls + Lsub],
                )

            lsoff = {c: 0 for c in range(NCT)}
            for c, j in order:
                LHC = CLIST[j]
                ls = lsoff[c]

                sf = gates_pool.tile([P, LHC], F16, tag="sf")
                si = gates_pool.tile([P, LHC], F16, tag="si")
                sh = gates_pool.tile([P, LHC], F16, tag="sh")
                fp = gates_pool.tile([P, LHC], F16, tag="fp")

                def mms8(ps, ocol):
                    for j2 in range(LHC // LT):
                        xk = slice(ls + j2 * LT, ls + (j2 + 1) * LT)
                        jl = slice(j2 * LT, (j2 + 1) * LT)
                        for kp in range(NK // 2):
                            nc.tensor.matmul(
                                ps[:, jl],
                                w8_sb[:, 2 * kp : 2 * kp + 2, ocol : ocol + P],
                                xT8_sb[:, 2 * kp : 2 * kp + 2, xk],
                                start=kp == 0,
                                stop=kp == NK // 2 - 1,
                                perf_mode=DR,
                            )

                def mmsbf(ps, ocol):
                    for j2 in range(LHC // LT):
                        xk = slice(ls + j2 * LT, ls + (j2 + 1) * LT)
                        jl = slice(j2 * LT, (j2 + 1) * LT)
                        for ki in range(NK):
                            nc.tensor.matmul(
                                ps[:, jl],
                                wbf_sb[:, ki, ocol : ocol + P],
                                xTbf_sb[:, ki, xk],
                                start=ki == 0,
                                stop=ki == NK - 1,
                            )

                # F gate
                psF = psum_pool.tile([P, LHC], F32, tag="ps")
                mms8(psF, c * CW8)
                nc.scalar.activation(
                    sf[:], psF[:], Act.Sigmoid,
                    bias=bias_sb[:, 0 * NCT + c : 0 * NCT + c + 1],
                    scale=1.0 / (XS * WS),
                )
                # I gate
                psI = psum_pool.tile([P, LHC], F32, tag="ps")
                mms8(psI, c * CW8 + P)
                nc.scalar.activation(
                    si[:], psI[:], Act.Sigmoid,
                    bias=bias_sb[:, 1 * NCT + c : 1 * NCT + c + 1],
                    scale=1.0 / (XS * WS),
                )
                # H gate
                psH = psum_pool.tile([P, LHC], F32, tag="ps")
                mmsbf(psH, c * P)
                nc.scalar.activation(
                    sh[:], psH[:], Act.Sigmoid,
                    bias=bias_sb[:, 2 * NCT + c : 2 * NCT + c + 1],
                )
                # Route selection: first round (small chunks) combines g on
                # DVE directly (max-route, relu bias b_h+0.5) - the SWDGE
                # round-trip latency would dominate the short fill-phase
                # slots. Steady-state big chunks use the min+SWDGE-add route
                # (relu bias b_h) to keep DVE lean.
                swdge_route = j > 0
                bcol = (2 if swdge_route else 3) * NCT + c
                rl = gates_pool.tile([P, LHC], F16, tag="rl")
                nc.scalar.activation(
                    rl[:], psH[:], Act.Relu,
                    bias=bias_sb[:, bcol : bcol + 1],
                )
                # f' = sf/(sf+si), one fused DVE op over the whole chunk
                nc.vector._custom_dve(
                    fract_op, out=fp[:], in0=sf[:], in1=si[:],
                    s0=FR_C0, s1=FR_C1,
                )
                # htil: min-route = relu(psH+b_h) + min(sigmoid, 0.5) with
                # the add as a SWDGE accumulate DMA; max-route =
                # max(relu(psH+b_h+0.5), sigmoid) in-place on DVE (2x mode).
                if swdge_route:
                    mn = gates_pool.tile([P, LHC], F16, tag="mn")
                    nc.vector.tensor_scalar_min(mn[:], sh[:], 0.5)
                    nc.gpsimd.dma_start(out=rl[:], in_=mn[:], accum_op=Alu.add)
                else:
                    nc.vector.tensor_tensor(rl[:], rl[:], sh[:], Alu.max)
                fpm1 = gates_pool.tile([P, LHC], F16, tag="fpm1")
                nc.vector.tensor_scalar_add(fpm1[:], fp[:], -1.0)

                # Emit chunk k-2's sub-scans, then chunk k-1's wv multiply
                # (the SWDGE-add round trip needs a slot of slack before the
                # DVE mult reads rl).
                while len(pend_scans) > 4:
                    emit_subscan(pend_scans.pop(0))
                pend_wv.append((c, ls, LHC, fp, fpm1, rl))
                if len(pend_wv) > 2:
                    emit_wv(pend_wv.pop(0))
                lsoff[c] += LHC

            while pend_wv:
                emit_wv(pend_wv.pop(0))
            while pend_scans:
                emit_subscan(pend_scans.pop(0))

    nc.compile()
    _cached_nc[key] = nc
    return nc


def reorder_w8(W: np.ndarray) -> np.ndarray:
    """fp8 weights, [H_IN, NCT*2P] with per-c-tile [F_c | I_c] col groups."""
    Wf, Wi = W[:H], W[H : 2 * H]
    cols = []
    for c in range(NCT):
        cols.append(Wf[c * P : (c + 1) * P])
        cols.append(Wi[c * P : (c + 1) * P])
    w = np.concatenate(cols, axis=0)  # [NCT*2P, H_IN]
    w8 = np.clip(w.T * WS, -240.0, 240.0).astype(ml_dtypes.float8_e4m3fn)
    return np.ascontiguousarray(w8)


def reorder_w16(W: np.ndarray) -> np.ndarray:
    Wh = W[2 * H :]
    return np.ascontiguousarray(Wh.T.astype(ml_dtypes.bfloat16))


def make_bias(b: np.ndarray) -> np.ndarray:
    b32 = np.asarray(b, dtype=np.float32)
    bias = np.empty((P, 16), dtype=np.float32)
    for c in range(NCT):
        bias[:, 0 * NCT + c] = b32[0 * H + c * P : 0 * H + (c + 1) * P]
        bias[:, 1 * NCT + c] = b32[1 * H + c * P : 1 * H + (c + 1) * P]
        bias[:, 2 * NCT + c] = b32[2 * H + c * P : 2 * H + (c + 1) * P]
        bias[:, 3 * NCT + c] = b32[2 * H + c * P : 2 * H + (c + 1) * P] + 0.5
    return bias


def _prep_core_inputs(x_n, wbf, w8, bias):
    xT = np.ascontiguousarray(x_n.T)
    return {
        "xTbf": xT.astype(ml_dtypes.bfloat16),
        "xT8": np.clip(xT * XS, -240.0, 240.0).astype(ml_dtypes.float8_e4m3fn),
        "wbf": wbf,
        "w8": w8,
        "bias": bias,
    }


def kernel(x: np.ndarray, W: np.ndarray, b: np.ndarray) -> np.ndarray:
    from concourse.bass_utils import run_bass_kernel_spmd

    nc = build_program()

    W = np.asarray(W)
    wbf = reorder_w16(W)
    w8 = reorder_w8(W)
    bias = make_bias(b)

    in_maps = [_prep_core_inputs(np.asarray(x[n]), wbf, w8, bias) for n in range(N)]
    res = run_bass_kernel_spmd(nc, in_maps, list(range(N)))

    out = np.empty((N, L, H), dtype=np.float32)
    for n in range(N):
        out[n] = res.results[n]["out"].T.astype(np.float32)
    return out
